# revision 10
# baseline (speedup 1.0000x reference)
"""Trainium2 Bass kernel for nn_CausalSelfAttention (B=2, T=4096, D=512, H=8, hd=64).

Sharding: batch x head-pair over 8 cores (core i: batch i//4, heads 2*(i%4), 2*(i%4)+1).
Each core computes QKV projection + RoPE + full-T causal attention for its 2 heads and
a partial output projection (row-parallel c_proj); host sums the 4 partials per batch.

Per-core dataflow (all [partition, free] SBUF layouts, T on the free dim):
  xT [512, T] --matmul--> qT/kT/vT [128, T] (PSUM)
  RoPE: 2 DVE muls with [cos;sin] / [-sin;cos] row-tiles + cross-quadrant adds
  per head h, q-block J (512 wide): S^T chunks [128k, 512q] = krT.T @ qrT (fp32r)
  exp via ACT (scale=1/8 folded in, no max subtraction - scores are O(5)), bf16 out
  causal diag masked by bf16 multiply; AV with ones-augmented V gives o and l=sum(exp)
  normalize by 1/l (DVE recip + gpsimd partition broadcast), then y^T = Wp_slice @ o
"""

import sys

sys.path.insert(0, "/opt/trn_rl_repo")

from contextlib import ExitStack

import ml_dtypes
import numpy as np

import concourse.bass as bass
import concourse.tile as tile
from concourse import bacc, mybir
from concourse.bass import ts
from concourse.bass_utils import run_bass_kernel_spmd

F32 = mybir.dt.float32
F32R = mybir.dt.float32r
BF16 = mybir.dt.bfloat16

B, C, H, HD = 2, 512, 8, 64
N_CORES = 8


def r32(ap):
    return ap.bitcast(F32R)


def build_kernel(T=4096, n_cores=N_CORES, dbg=False):
    nc = bacc.Bacc(
        "TRN2",
        target_bir_lowering=False,
        debug=False,
        num_devices=n_cores,
    )
    NJ = T // 512
    NK = T // 128

    xT_d = nc.dram_tensor("xT", [C, T], F32R, kind="ExternalInput").ap()
    cs1_d = nc.dram_tensor("cs1T", [128, T], F32, kind="ExternalInput").ap()
    cs2_d = nc.dram_tensor("cs2T", [128, T], F32, kind="ExternalInput").ap()
    wq_d = nc.dram_tensor("wqT", [C, 128], F32R, kind="ExternalInput").ap()
    wk_d = nc.dram_tensor("wkT", [C, 128], F32R, kind="ExternalInput").ap()
    wv_d = nc.dram_tensor("wvT", [C, 128], F32R, kind="ExternalInput").ap()
    wp_d = nc.dram_tensor("wpT", [128, C], F32R, kind="ExternalInput").ap()
    msk_d = nc.dram_tensor("masks", [128, 4, 512], BF16, kind="ExternalInput").ap()
    id_d = nc.dram_tensor("ident2", [128, 64], BF16, kind="ExternalInput").ap()
    y_d = nc.dram_tensor("yT", [C, T], F32, kind="ExternalOutput").ap()
    if dbg:
        dbg_qr = nc.dram_tensor("dbg_qr", [128, T], F32R, kind="ExternalOutput").ap()
        dbg_kr = nc.dram_tensor("dbg_kr", [128, T], F32R, kind="ExternalOutput").ap()
        dbg_va = nc.dram_tensor("dbg_va", [128, 2, NK, 65], F32, kind="ExternalOutput").ap()
        dbg_o = nc.dram_tensor("dbg_o", [128, T], F32R, kind="ExternalOutput").ap()

    SCALE = float(1.0 / np.sqrt(HD))

    with tile.TileContext(nc) as tc, ExitStack() as ctx:
        consts = ctx.enter_context(tc.tile_pool(name="consts", bufs=1))
        big = ctx.enter_context(tc.tile_pool(name="big", bufs=1))
        xpool = ctx.enter_context(tc.tile_pool(name="xpool", bufs=8))
        mpool = ctx.enter_context(tc.tile_pool(name="mpool", bufs=3))
        epool = ctx.enter_context(tc.tile_pool(name="epool", bufs=3))
        spool = ctx.enter_context(tc.tile_pool(name="small", bufs=2))
        ypool = ctx.enter_context(tc.tile_pool(name="ypool", bufs=4))

        cs1 = consts.tile([128, T], F32)
        nc.sync.dma_start(cs1[:], cs1_d[:])
        cs2 = consts.tile([128, T], F32)
        nc.sync.dma_start(cs2[:], cs2_d[:])
        w_q = consts.tile([128, 4, 128], F32R)
        nc.sync.dma_start(w_q[:], wq_d.rearrange("(c p) m -> p c m", c=4))
        w_k = consts.tile([128, 4, 128], F32R)
        nc.sync.dma_start(w_k[:], wk_d.rearrange("(c p) m -> p c m", c=4))
        w_v = consts.tile([128, 4, 128], F32R)
        nc.sync.dma_start(w_v[:], wv_d.rearrange("(c p) m -> p c m", c=4))
        w_p = consts.tile([128, C], F32R)
        nc.sync.dma_start(w_p[:], wp_d[:])
        masks = consts.tile([128, 4, 512], BF16)
        nc.sync.dma_start(masks[:], msk_d[:])
        ident2 = consts.tile([128, 64], BF16)
        nc.sync.dma_start(ident2[:], id_d[:])

        qrT = big.tile([128, T], F32R)
        krT = big.tile([128, T], F32R)
        vTb = big.tile([128, T], BF16)
        oTf = big.tile([128, T], F32R)
        v_aug = big.tile([128, 2, NK, 65], BF16)
        nc.gpsimd.memset(v_aug[:], 1.0)

        # ======== Phase 1: QKV projection + rope + v transpose (per j) ========
        with tc.tile_pool(name="ps_mm", bufs=6, space="PSUM") as ps_mm:
            for j in range(NJ):
                jc = ts(j, 512)
                xc = []
                for c in range(4):
                    xt = xpool.tile([128, 512], F32R, tag="xc")
                    nc.sync.dma_start(xt[:], xT_d[ts(c, 128), jc])
                    xc.append(xt)
                q_ps = ps_mm.tile([128, 512], F32, tag="mm")
                k_ps = ps_mm.tile([128, 512], F32, tag="mm")
                v_ps = ps_mm.tile([128, 512], F32, tag="mm")
                for c in range(4):
                    st, sp = c == 0, c == 3
                    nc.tensor.matmul(q_ps[:], w_q[:, c, :], xc[c][:], start=st, stop=sp)
                    nc.tensor.matmul(k_ps[:], w_k[:, c, :], xc[c][:], start=st, stop=sp)
                    nc.tensor.matmul(v_ps[:], w_v[:, c, :], xc[c][:], start=st, stop=sp)
                # rope: muls read PSUM; cross-quadrant single-input copies align
                # the half-blocks, then same-base adds (two-input SBUF ops must
                # share a base partition on DVE)
                for z_ps, zrT in ((q_ps, qrT), (k_ps, krT)):
                    m1 = mpool.tile([128, 512], F32, tag="m1")
                    m2 = mpool.tile([128, 512], F32, tag="m2")
                    mc = mpool.tile([128, 512], F32, tag="mc")
                    nc.vector.tensor_mul(m1[:], z_ps[:], cs1[:, jc])
                    nc.vector.tensor_mul(m2[:], z_ps[:], cs2[:, jc])
                    for h in range(2):
                        r = 64 * h
                        # mc[r:r+32] <- m1[r+32:r+64]; mc[r+32:r+64] <- m2[r:r+32]
                        nc.vector.tensor_copy(mc[r : r + 32, :], m1[r + 32 : r + 64, :])
                        nc.vector.tensor_copy(mc[r + 32 : r + 64, :], m2[r : r + 32, :])
                        nc.vector.tensor_add(zrT[r : r + 32, jc], m1[r : r + 32, :], mc[r : r + 32, :])
                        nc.vector.tensor_add(zrT[r + 32 : r + 64, jc], m2[r + 32 : r + 64, :], mc[r + 32 : r + 64, :])
                nc.scalar.copy(vTb[:, jc], v_ps[:])
                # v transpose for this j's 4 chunks
                for h in range(2):
                    rr = 64 * h
                    for ci in range(4):
                        cc = 4 * j + ci
                        vt_ps = ps_mm.tile([128, 64], BF16, tag="mm")
                        nc.tensor.transpose(vt_ps[:], vTb[rr : rr + 64, ts(cc, 128)], ident2[rr : rr + 64, :])
                        nc.vector.tensor_copy(v_aug[:, h, cc, 0:64], vt_ps[:])

        # ======== Phase 2: attention (J outer, head inner) + y proj per J ========
        with (
            tc.tile_pool(name="ps_s", bufs=2, space="PSUM") as ps_s,
            tc.tile_pool(name="ps_o", bufs=1, space="PSUM") as ps_o,
            tc.tile_pool(name="ps_y", bufs=1, space="PSUM") as ps_y,
        ):
            for J in range(NJ):
                jc = ts(J, 512)
                for h in range(2):
                    r = 64 * h
                    nchunks = 4 * J + 4
                    o_ps = ps_o.tile([65, 512], F32, tag="o")
                    ngroups = (nchunks + 2) // 3
                    for g in range(ngroups):
                        c0 = 3 * g
                        gn = min(3, nchunks - c0)
                        s_ps = ps_s.tile([128, 1536], F32, tag="s")
                        for ci in range(gn):
                            cc = c0 + ci
                            nc.tensor.matmul(
                                s_ps[:, ts(ci, 512)],
                                krT[r : r + 64, ts(cc, 128)],
                                qrT[r : r + 64, jc],
                                start=True,
                                stop=True,
                            )
                        e_sb = epool.tile([128, 1536], BF16, tag="e")
                        nc.scalar.activation(
                            e_sb[:, 0 : 512 * gn],
                            s_ps[:, 0 : 512 * gn],
                            mybir.ActivationFunctionType.Exp,
                            scale=SCALE,
                        )
                        for ci in range(gn):
                            cc = c0 + ci
                            m = cc - 4 * J
                            if m >= 0:
                                nc.vector.tensor_mul(
                                    e_sb[:, ts(ci, 512)], e_sb[:, ts(ci, 512)], masks[:, m, :]
                                )
                            nc.tensor.matmul(
                                o_ps[:],
                                v_aug[:, h, cc, :],
                                e_sb[:, ts(ci, 512)],
                                start=(cc == 0),
                                stop=(cc == nchunks - 1),
                            )
                    # normalize: oTf[r:r+64, jc] = o_ps[0:64] * (1/l)
                    rl = spool.tile([1, 512], F32, tag="rl")
                    nc.vector.reciprocal(rl[:], o_ps[64:65, :])
                    bc = spool.tile([64, 512], F32, tag="bc")
                    nc.gpsimd.partition_broadcast(bc[:], rl[:])
                    nc.vector.tensor_mul(oTf[r : r + 64, jc], o_ps[0:64, :], bc[:])

                # y projection for this t-block (needs both heads of J)
                for c in range(4):
                    y_ps = ps_y.tile([128, 512], F32, tag="y")
                    nc.tensor.matmul(
                        y_ps[:], w_p[:, ts(c, 128)], oTf[:, jc], start=True, stop=True
                    )
                    y_sb = ypool.tile([128, 512], F32, tag="ysb")
                    if c % 2 == 0:
                        nc.vector.tensor_copy(y_sb[:], y_ps[:])
                    else:
                        nc.scalar.copy(y_sb[:], y_ps[:])
                    nc.sync.dma_start(y_d[ts(c, 128), jc], y_sb[:])

            if dbg:
                nc.sync.dma_start(dbg_qr[:], qrT[:])
                nc.sync.dma_start(dbg_kr[:], krT[:])
                va32 = big.tile([128, 2, NK, 65], F32)
                nc.vector.tensor_copy(va32[:], v_aug[:])
                nc.sync.dma_start(dbg_va[:], va32[:])
                nc.sync.dma_start(dbg_o[:], oTf[:])

    nc.compile()
    return nc


# ---------------- host-side wrapper ----------------

_CACHE = {}


def _get_nc(T):
    if T not in _CACHE:
        _CACHE[T] = build_kernel(T)
    return _CACHE[T]


def _host_prep(x, cos, sin, Wq, Wk, Wv, Wp):
    T = x.shape[1]
    cosT = np.ascontiguousarray(cos.T).astype(np.float32)
    sinT = np.ascontiguousarray(sin.T).astype(np.float32)
    cs1T = np.concatenate([cosT, sinT, cosT, sinT], axis=0)
    cs2T = np.concatenate([-sinT, cosT, -sinT, cosT], axis=0)
    rr = np.arange(128)[:, None]
    cq = np.arange(512)[None, :]
    masks = np.stack(
        [(cq >= 128 * m + rr) for m in range(4)], axis=1
    ).astype(ml_dtypes.bfloat16)  # [128, 4, 512]
    ident2 = np.concatenate([np.eye(64), np.eye(64)], axis=0).astype(ml_dtypes.bfloat16)

    in_maps = []
    for core in range(N_CORES):
        b, p = core // 4, core % 4
        hs = slice(128 * p, 128 * (p + 1))
        in_maps.append(
            {
                "xT": np.ascontiguousarray(x[b].T.astype(np.float32)),
                "cs1T": cs1T,
                "cs2T": cs2T,
                "wqT": np.ascontiguousarray(Wq[hs].T.astype(np.float32)),
                "wkT": np.ascontiguousarray(Wk[hs].T.astype(np.float32)),
                "wvT": np.ascontiguousarray(Wv[hs].T.astype(np.float32)),
                "wpT": np.ascontiguousarray(Wp[:, hs].T.astype(np.float32)),
                "masks": masks,
                "ident2": ident2,
            }
        )
    return in_maps


def kernel(x, cos, sin, Wq, Wk, Wv, Wp, _trace=False, _nc=None):
    x = np.asarray(x)
    T = x.shape[1]
    nc = _nc if _nc is not None else _get_nc(T)
    in_maps = _host_prep(
        x, np.asarray(cos), np.asarray(sin),
        np.asarray(Wq), np.asarray(Wk), np.asarray(Wv), np.asarray(Wp),
    )
    res = run_bass_kernel_spmd(nc, in_maps, list(range(N_CORES)), trace=_trace)
    y = np.zeros((B, T, C), np.float32)
    for core in range(N_CORES):
        y[core // 4] += res.results[core]["yT"].T
    kernel.last_results = res
    return y


# revision 13
# speedup vs baseline: 1.4384x; 1.4384x over previous
"""Trainium2 Bass kernel for nn_CausalSelfAttention (B=2, T=4096, D=512, H=8, hd=64).

Sharding: batch x head-pair over 8 cores (core i: batch i//4, heads 2*(i%4), 2*(i%4)+1).
Each core computes QKV projection + RoPE + full-T causal attention for its 2 heads and
a partial output projection (row-parallel c_proj); host sums the 4 partials per batch.

Per-core dataflow (all [partition, free] SBUF layouts, T on the free dim):
  xT [512, T] --matmul--> qA/qB/kA/kB/vT [128, T] (PSUM), where the B-projections
  use host-side half-swapped-and-negated weight rows so RoPE reduces to
  qr = qA*[cos] + qB*[sin] - three full-width same-partition DVE ops per tile.
  per head h, q-block J (512 wide): S^T chunks [128k, 512q] = krT.T @ qrT (fp32r)
  exp via ACT (scale=1/8 folded in, no max subtraction - scores are O(5)), bf16 out
  causal diag masked by bf16 multiply; AV with ones-augmented V gives o and l=sum(exp)
  normalize by 1/l (fast DVE recip + gpsimd partition broadcast), then yT = WpT.T @ o
"""

import sys

sys.path.insert(0, "/opt/trn_rl_repo")

from contextlib import ExitStack

import ml_dtypes
import numpy as np

import concourse.bass as bass
import concourse.tile as tile
from concourse import bacc, mybir
from concourse.bass import ts
from concourse.bass_utils import run_bass_kernel_spmd

F32 = mybir.dt.float32
F32R = mybir.dt.float32r
BF16 = mybir.dt.bfloat16

B, C, H, HD = 2, 512, 8, 64
N_CORES = 8


def build_kernel(T=4096, n_cores=N_CORES, dbg=False):
    nc = bacc.Bacc(
        "TRN2",
        target_bir_lowering=False,
        debug=False,
        num_devices=n_cores,
    )
    NJ = T // 512
    NK = T // 128

    xT_d = nc.dram_tensor("xT", [C, T], F32R, kind="ExternalInput").ap()
    cc_d = nc.dram_tensor("ccT", [128, T], F32, kind="ExternalInput").ap()
    ss_d = nc.dram_tensor("ssT", [128, T], F32, kind="ExternalInput").ap()
    w_d = {}
    for name in ("wqT", "wqbT", "wkT", "wkbT", "wvT"):
        w_d[name] = nc.dram_tensor(name, [C, 128], F32R, kind="ExternalInput").ap()
    wp_d = nc.dram_tensor("wpT", [128, C], F32R, kind="ExternalInput").ap()
    msk_d = nc.dram_tensor("masks", [128, 4, 512], BF16, kind="ExternalInput").ap()
    id_d = nc.dram_tensor("ident2", [128, 64], BF16, kind="ExternalInput").ap()
    y_d = nc.dram_tensor("yT", [C, T], F32, kind="ExternalOutput").ap()
    if dbg:
        dbg_qr = nc.dram_tensor("dbg_qr", [128, T], F32R, kind="ExternalOutput").ap()
        dbg_kr = nc.dram_tensor("dbg_kr", [128, T], F32R, kind="ExternalOutput").ap()
        dbg_va = nc.dram_tensor("dbg_va", [128, 2, NK, 65], F32, kind="ExternalOutput").ap()
        dbg_o = nc.dram_tensor("dbg_o", [128, T], F32R, kind="ExternalOutput").ap()

    SCALE = float(1.0 / np.sqrt(HD))

    with tile.TileContext(nc) as tc, ExitStack() as ctx:
        consts = ctx.enter_context(tc.tile_pool(name="consts", bufs=1))
        big = ctx.enter_context(tc.tile_pool(name="big", bufs=1))
        xpool = ctx.enter_context(tc.tile_pool(name="xpool", bufs=8))
        mpool = ctx.enter_context(tc.tile_pool(name="mpool", bufs=3))
        epool = ctx.enter_context(tc.tile_pool(name="epool", bufs=4))
        spool = ctx.enter_context(tc.tile_pool(name="small", bufs=2))
        ypool = ctx.enter_context(tc.tile_pool(name="ypool", bufs=4))

        cc = consts.tile([128, T], F32)
        nc.sync.dma_start(cc[:], cc_d[:])
        ss = consts.tile([128, T], F32)
        nc.sync.dma_start(ss[:], ss_d[:])
        w_sb = {}
        for name in ("wqT", "wqbT", "wkT", "wkbT", "wvT"):
            w = consts.tile([128, 4, 128], F32R, tag=name, name=f"w_{name}")
            nc.sync.dma_start(w[:], w_d[name].rearrange("(c p) m -> p c m", c=4))
            w_sb[name] = w
        w_p = consts.tile([128, C], F32R)
        nc.sync.dma_start(w_p[:], wp_d[:])
        masks = consts.tile([128, 4, 512], BF16)
        nc.sync.dma_start(masks[:], msk_d[:])
        ident2 = consts.tile([128, 64], BF16)
        nc.sync.dma_start(ident2[:], id_d[:])

        qrT = big.tile([128, T], F32R)
        krT = big.tile([128, T], F32R)
        vTb = big.tile([128, T], BF16)
        oTf = big.tile([128, T], F32R)
        v_aug = big.tile([128, 2, NK, 65], BF16)
        nc.gpsimd.memset(v_aug[:], 1.0)

        # ======== Phase 1: projections + rope + v transpose (per j) ========
        with tc.tile_pool(name="ps_mm", bufs=7, space="PSUM") as ps_mm:
            for j in range(NJ):
                jc = ts(j, 512)
                xc = []
                for c in range(4):
                    xt = xpool.tile([128, 512], F32R, tag="xc")
                    nc.sync.dma_start(xt[:], xT_d[ts(c, 128), jc])
                    xc.append(xt)
                ps = {}
                for name in ("wqT", "wqbT", "wkT", "wkbT", "wvT"):
                    ps[name] = ps_mm.tile([128, 512], F32, tag="mm", name=f"ps_{name}_{j}")
                for c in range(4):
                    st, sp = c == 0, c == 3
                    for name in ("wqT", "wqbT", "wkT", "wkbT", "wvT"):
                        nc.tensor.matmul(ps[name][:], w_sb[name][:, c, :], xc[c][:], start=st, stop=sp)
                # rope: qr = qA*cc + qB*ss, full-width same-partition ops
                for a, b, zrT in (("wqT", "wqbT", qrT), ("wkT", "wkbT", krT)):
                    m1 = mpool.tile([128, 512], F32, tag="m1")
                    m2 = mpool.tile([128, 512], F32, tag="m2")
                    nc.vector.tensor_mul(m1[:], ps[a][:], cc[:, jc])
                    nc.vector.tensor_mul(m2[:], ps[b][:], ss[:, jc])
                    nc.vector.tensor_add(zrT[:, jc], m1[:], m2[:])
                nc.scalar.copy(vTb[:, jc], ps["wvT"][:])
                # v transpose for this j's 4 chunks
                for h in range(2):
                    rr = 64 * h
                    for ci in range(4):
                        cci = 4 * j + ci
                        vt_ps = ps_mm.tile([128, 64], BF16, tag="mm")
                        nc.tensor.transpose(vt_ps[:], vTb[rr : rr + 64, ts(cci, 128)], ident2[rr : rr + 64, :])
                        nc.scalar.copy(v_aug[:, h, cci, 0:64], vt_ps[:])

        # ======== Phase 2: attention (J outer, head inner) + y proj per J ========
        with (
            tc.tile_pool(name="ps_s", bufs=2, space="PSUM") as ps_s,
            tc.tile_pool(name="ps_o", bufs=1, space="PSUM") as ps_o,
            tc.tile_pool(name="ps_y", bufs=1, space="PSUM") as ps_y,
        ):
            for J in range(NJ):
                jc = ts(J, 512)
                for h in range(2):
                    r = 64 * h
                    nchunks = 4 * J + 4
                    o_ps = ps_o.tile([65, 512], F32, tag="o")
                    ngroups = (nchunks + 2) // 3
                    for g in range(ngroups):
                        c0 = 3 * g
                        gn = min(3, nchunks - c0)
                        s_ps = ps_s.tile([128, 1536], F32, tag="s")
                        for ci in range(gn):
                            cci = c0 + ci
                            nc.tensor.matmul(
                                s_ps[:, ts(ci, 512)],
                                krT[r : r + 64, ts(cci, 128)],
                                qrT[r : r + 64, jc],
                                start=True,
                                stop=True,
                            )
                        e_sb = epool.tile([128, 1536], BF16, tag="e")
                        nc.scalar.activation(
                            e_sb[:, 0 : 512 * gn],
                            s_ps[:, 0 : 512 * gn],
                            mybir.ActivationFunctionType.Exp,
                            scale=SCALE,
                        )
                        for ci in range(gn):
                            cci = c0 + ci
                            m = cci - 4 * J
                            if m >= 0:
                                nc.vector.tensor_mul(
                                    e_sb[:, ts(ci, 512)], e_sb[:, ts(ci, 512)], masks[:, m, :]
                                )
                            nc.tensor.matmul(
                                o_ps[:],
                                v_aug[:, h, cci, :],
                                e_sb[:, ts(ci, 512)],
                                start=(cci == 0),
                                stop=(cci == nchunks - 1),
                            )
                    # normalize: oTf[r:r+64, jc] = o_ps[0:64] * (1/l)
                    # (recip_approx_fast needs SBUF input + multi-partition:
                    # copy l out of PSUM, broadcast, then invert on [64, 512])
                    l_sb = spool.tile([1, 512], F32, tag="lsb")
                    nc.vector.tensor_copy(l_sb[:], o_ps[64:65, :])
                    bc = spool.tile([64, 512], F32, tag="bc")
                    nc.gpsimd.partition_broadcast(bc[:], l_sb[:])
                    rb = spool.tile([64, 512], F32, tag="rb")
                    nc.vector.reciprocal_approx_fast(rb[:], bc[:])
                    nc.vector.tensor_mul(oTf[r : r + 64, jc], o_ps[0:64, :], rb[:])

                # y projection for this t-block (needs both heads of J)
                for c in range(4):
                    y_ps = ps_y.tile([128, 512], F32, tag="y")
                    nc.tensor.matmul(
                        y_ps[:], w_p[:, ts(c, 128)], oTf[:, jc], start=True, stop=True
                    )
                    y_sb = ypool.tile([128, 512], F32, tag="ysb")
                    nc.vector.tensor_copy(y_sb[:], y_ps[:])
                    nc.sync.dma_start(y_d[ts(c, 128), jc], y_sb[:])

            if dbg:
                nc.sync.dma_start(dbg_qr[:], qrT[:])
                nc.sync.dma_start(dbg_kr[:], krT[:])
                va32 = big.tile([128, 2, NK, 65], F32)
                nc.vector.tensor_copy(va32[:], v_aug[:])
                nc.sync.dma_start(dbg_va[:], va32[:])
                nc.sync.dma_start(dbg_o[:], oTf[:])

    nc.compile()
    return nc


# ---------------- host-side wrapper ----------------

_CACHE = {}


def _get_nc(T):
    if T not in _CACHE:
        _CACHE[T] = build_kernel(T)
    return _CACHE[T]


def _swap_neg_rows(W):
    """Per 64-row head block of W [128, 512]: rows -> [W[32:64], -W[0:32]]."""
    out = np.empty_like(W)
    for h in range(2):
        r = 64 * h
        out[r : r + 32] = W[r + 32 : r + 64]
        out[r + 32 : r + 64] = -W[r : r + 32]
    return out


def _host_prep(x, cos, sin, Wq, Wk, Wv, Wp):
    T = x.shape[1]
    cosT = np.ascontiguousarray(cos.T).astype(np.float32)  # [32, T]
    sinT = np.ascontiguousarray(sin.T).astype(np.float32)
    ccT = np.concatenate([cosT] * 4, axis=0)  # [128, T]
    ssT = np.concatenate([sinT] * 4, axis=0)
    rr = np.arange(128)[:, None]
    cq = np.arange(512)[None, :]
    masks = np.stack(
        [(cq >= 128 * m + rr) for m in range(4)], axis=1
    ).astype(ml_dtypes.bfloat16)  # [128, 4, 512]
    ident2 = np.concatenate([np.eye(64), np.eye(64)], axis=0).astype(ml_dtypes.bfloat16)

    in_maps = []
    for core in range(N_CORES):
        b, p = core // 4, core % 4
        hs = slice(128 * p, 128 * (p + 1))
        Wq_h = Wq[hs].astype(np.float32)  # [128, 512]
        Wk_h = Wk[hs].astype(np.float32)
        in_maps.append(
            {
                "xT": np.ascontiguousarray(x[b].T.astype(np.float32)),
                "ccT": ccT,
                "ssT": ssT,
                "wqT": np.ascontiguousarray(Wq_h.T),
                "wqbT": np.ascontiguousarray(_swap_neg_rows(Wq_h).T),
                "wkT": np.ascontiguousarray(Wk_h.T),
                "wkbT": np.ascontiguousarray(_swap_neg_rows(Wk_h).T),
                "wvT": np.ascontiguousarray(Wv[hs].T.astype(np.float32)),
                "wpT": np.ascontiguousarray(Wp[:, hs].T.astype(np.float32)),
                "masks": masks,
                "ident2": ident2,
            }
        )
    return in_maps


def kernel(x, cos, sin, Wq, Wk, Wv, Wp, _trace=False, _nc=None):
    x = np.asarray(x)
    T = x.shape[1]
    nc = _nc if _nc is not None else _get_nc(T)
    in_maps = _host_prep(
        x, np.asarray(cos), np.asarray(sin),
        np.asarray(Wq), np.asarray(Wk), np.asarray(Wv), np.asarray(Wp),
    )
    res = run_bass_kernel_spmd(nc, in_maps, list(range(N_CORES)), trace=_trace)
    y = np.zeros((B, T, C), np.float32)
    for core in range(N_CORES):
        y[core // 4] += res.results[core]["yT"].T
    kernel.last_results = res
    return y


# revision 16
# speedup vs baseline: 1.6340x; 1.1360x over previous
"""Trainium2 Bass kernel for nn_CausalSelfAttention (B=2, T=4096, D=512, H=8, hd=64).

Sharding: batch x head-pair over 8 cores (core i: batch i//4, heads 2*(i%4), 2*(i%4)+1).
Each core computes QKV projection + RoPE + full-T causal attention for its 2 heads and
a partial output projection (row-parallel c_proj); host sums the 4 partials per batch.

Per-core dataflow (all [partition, free] SBUF layouts, T on the free dim):
  xT [512, T] --matmul--> qA/qB/kA/kB/vT [128, T] (PSUM), where the B-projections
  use host-side half-swapped-and-negated weight rows so RoPE reduces to
  qr = qA*[cos] + qB*[sin] - three full-width same-partition DVE ops per tile.
  per head h, q-block J (512 wide): S^T chunks [128k, 512q] = krT.T @ qrT (fp32r)
  exp via ACT (scale=1/8 folded in, no max subtraction - scores are O(5)), bf16 out
  causal diag masked by bf16 multiply; AV with ones-augmented V gives o and l=sum(exp)
  normalize by 1/l (fast DVE recip + gpsimd partition broadcast), then yT = WpT.T @ o
"""

import sys

sys.path.insert(0, "/opt/trn_rl_repo")

from contextlib import ExitStack

import ml_dtypes
import numpy as np

import concourse.bass as bass
import concourse.tile as tile
from concourse import bacc, mybir
from concourse.bass import ts
from concourse.bass_utils import run_bass_kernel_spmd

F32 = mybir.dt.float32
F32R = mybir.dt.float32r
F16 = mybir.dt.float16
BF16 = mybir.dt.bfloat16

B, C, H, HD = 2, 512, 8, 64
N_CORES = 8


def build_kernel(T=4096, n_cores=N_CORES, dbg=False):
    nc = bacc.Bacc(
        "TRN2",
        target_bir_lowering=False,
        debug=False,
        num_devices=n_cores,
    )
    NJ = T // 512
    NK = T // 128

    xT_d = nc.dram_tensor("xT", [C, T], F16, kind="ExternalInput").ap()
    cc_d = nc.dram_tensor("ccT", [128, T], F32, kind="ExternalInput").ap()
    ss_d = nc.dram_tensor("ssT", [128, T], F32, kind="ExternalInput").ap()
    w_d = {}
    for name in ("wqT", "wqbT", "wkT", "wkbT", "wvT"):
        w_d[name] = nc.dram_tensor(name, [C, 128], F16, kind="ExternalInput").ap()
    wp_d = nc.dram_tensor("wpT", [128, C], F16, kind="ExternalInput").ap()
    msk_d = nc.dram_tensor("masks", [128, 4, 512], F16, kind="ExternalInput").ap()
    id_d = nc.dram_tensor("ident2", [128, 64], F16, kind="ExternalInput").ap()
    y_d = nc.dram_tensor("yT", [C, T], F32, kind="ExternalOutput").ap()
    warm_d = nc.dram_tensor("warm", [1, 4], F32, kind="ExternalOutput").ap()
    if dbg:
        dbg_qr = nc.dram_tensor("dbg_qr", [128, T], F16, kind="ExternalOutput").ap()
        dbg_kr = nc.dram_tensor("dbg_kr", [128, T], F16, kind="ExternalOutput").ap()
        dbg_va = nc.dram_tensor("dbg_va", [128, 2, NK, 65], F32, kind="ExternalOutput").ap()
        dbg_o = nc.dram_tensor("dbg_o", [128, T], F16, kind="ExternalOutput").ap()

    SCALE = float(1.0 / np.sqrt(HD))

    with tile.TileContext(nc) as tc, ExitStack() as ctx:
        consts = ctx.enter_context(tc.tile_pool(name="consts", bufs=1))
        big = ctx.enter_context(tc.tile_pool(name="big", bufs=1))
        xpool = ctx.enter_context(tc.tile_pool(name="xpool", bufs=8))
        mpool = ctx.enter_context(tc.tile_pool(name="mpool", bufs=3))
        epool = ctx.enter_context(tc.tile_pool(name="epool", bufs=4))
        spool = ctx.enter_context(tc.tile_pool(name="small", bufs=2))
        ypool = ctx.enter_context(tc.tile_pool(name="ypool", bufs=4))

        w_sb = {}
        for name in ("wqT", "wqbT", "wkT", "wkbT", "wvT"):
            w = consts.tile([128, 4, 128], F16, tag=name, name=f"w_{name}")
            nc.sync.dma_start(w[:], w_d[name].rearrange("(c p) m -> p c m", c=4))
            w_sb[name] = w
        w_p = consts.tile([128, C], F16)
        nc.sync.dma_start(w_p[:], wp_d[:])
        masks = consts.tile([128, 4, 512], F16)
        nc.sync.dma_start(masks[:], msk_d[:])
        ident2 = consts.tile([128, 64], F16)
        nc.sync.dma_start(ident2[:], id_d[:])
        cc = consts.tile([128, T], F32)
        nc.gpsimd.dma_start(cc[:], cc_d[:])
        ss = consts.tile([128, T], F32)
        nc.gpsimd.dma_start(ss[:], ss_d[:])

        qrT = big.tile([128, T], F16)
        krT = big.tile([128, T], F16)
        vTb = big.tile([128, T], F16)
        oTf = big.tile([128, T], F16)
        v_aug = big.tile([128, 2, NK, 65], F16)
        nc.gpsimd.memset(v_aug[:], 1.0)

        # ======== Phase 1: projections + rope + v transpose (per j) ========
        with tc.tile_pool(name="ps_mm", bufs=7, space="PSUM") as ps_mm:
            # PE warmup burst: ~4us of matmuls on a memset tile so the HAM
            # clock gate releases (1.2 -> 2.4 GHz) before the real work.
            wz = xpool.tile([128, 512], F16, tag="wz")
            nc.gpsimd.memset(wz[:], 0.25)
            wu_ps = ps_mm.tile([128, 512], F32, tag="mm")
            for _ in range(10):
                nc.tensor.matmul(wu_ps[:], wz[:, 0:128], wz[:], start=True, stop=True)
            wsink = spool.tile([1, 4], F32, tag="wsink")
            nc.vector.tensor_copy(wsink[:], wu_ps[0:1, 0:4])
            nc.sync.dma_start(warm_d[:], wsink[:])
            for j in range(NJ):
                jc = ts(j, 512)
                xc = []
                for c in range(4):
                    xt = xpool.tile([128, 512], F16, tag="xc")
                    nc.sync.dma_start(xt[:], xT_d[ts(c, 128), jc])
                    xc.append(xt)
                ps = {}
                for name in ("wqT", "wqbT", "wkT", "wkbT", "wvT"):
                    ps[name] = ps_mm.tile([128, 512], F32, tag="mm", name=f"ps_{name}_{j}")
                for c in range(4):
                    st, sp = c == 0, c == 3
                    for name in ("wqT", "wqbT", "wkT", "wkbT", "wvT"):
                        nc.tensor.matmul(ps[name][:], w_sb[name][:, c, :], xc[c][:], start=st, stop=sp)
                # rope: qr = qA*cc + qB*ss, full-width same-partition ops
                for a, b, zrT in (("wqT", "wqbT", qrT), ("wkT", "wkbT", krT)):
                    m1 = mpool.tile([128, 512], F32, tag="m1")
                    m2 = mpool.tile([128, 512], F32, tag="m2")
                    nc.vector.tensor_mul(m1[:], ps[a][:], cc[:, jc])
                    nc.vector.tensor_mul(m2[:], ps[b][:], ss[:, jc])
                    nc.vector.tensor_add(zrT[:, jc], m1[:], m2[:])
                nc.scalar.copy(vTb[:, jc], ps["wvT"][:])
                # v transpose for this j's 4 chunks
                for h in range(2):
                    rr = 64 * h
                    for ci in range(4):
                        cci = 4 * j + ci
                        vt_ps = ps_mm.tile([128, 64], F16, tag="mm")
                        nc.tensor.transpose(vt_ps[:], vTb[rr : rr + 64, ts(cci, 128)], ident2[rr : rr + 64, :])
                        nc.scalar.copy(v_aug[:, h, cci, 0:64], vt_ps[:])

        # ======== Phase 2: attention (J outer, head inner) + y proj per J ========
        with (
            tc.tile_pool(name="ps_s", bufs=2, space="PSUM") as ps_s,
            tc.tile_pool(name="ps_o", bufs=1, space="PSUM") as ps_o,
            tc.tile_pool(name="ps_y", bufs=1, space="PSUM") as ps_y,
        ):
            for J in range(NJ):
                jc = ts(J, 512)
                for h in range(2):
                    r = 64 * h
                    nchunks = 4 * J + 4
                    o_ps = ps_o.tile([65, 512], F32, tag="o")
                    ngroups = (nchunks + 2) // 3
                    for g in range(ngroups):
                        c0 = 3 * g
                        gn = min(3, nchunks - c0)
                        s_ps = ps_s.tile([128, 1536], F32, tag="s")
                        for ci in range(gn):
                            cci = c0 + ci
                            nc.tensor.matmul(
                                s_ps[:, ts(ci, 512)],
                                krT[r : r + 64, ts(cci, 128)],
                                qrT[r : r + 64, jc],
                                start=True,
                                stop=True,
                            )
                        e_sb = epool.tile([128, 1536], F16, tag="e")
                        nc.scalar.activation(
                            e_sb[:, 0 : 512 * gn],
                            s_ps[:, 0 : 512 * gn],
                            mybir.ActivationFunctionType.Exp,
                            scale=SCALE,
                        )
                        for ci in range(gn):
                            cci = c0 + ci
                            m = cci - 4 * J
                            if m >= 0:
                                nc.vector.tensor_mul(
                                    e_sb[:, ts(ci, 512)], e_sb[:, ts(ci, 512)], masks[:, m, :]
                                )
                            nc.tensor.matmul(
                                o_ps[:],
                                v_aug[:, h, cci, :],
                                e_sb[:, ts(ci, 512)],
                                start=(cci == 0),
                                stop=(cci == nchunks - 1),
                            )
                    # normalize: oTf[r:r+64, jc] = o[0:64] * (1/l).
                    # One copy frees the PSUM accumulator immediately; the
                    # recip chain then runs from SBUF (recip_approx_fast needs
                    # SBUF input + multi-partition to behave on HW).
                    o_sb = spool.tile([64, 512], F32, tag="osb")
                    nc.vector.tensor_copy(o_sb[:], o_ps[0:64, :])
                    l_sb = spool.tile([1, 512], F32, tag="lsb")
                    nc.vector.tensor_copy(l_sb[:], o_ps[64:65, :])
                    bc = spool.tile([64, 512], F32, tag="bc")
                    nc.gpsimd.partition_broadcast(bc[:], l_sb[:])
                    rb = spool.tile([64, 512], F32, tag="rb")
                    nc.vector.reciprocal_approx_fast(rb[:], bc[:])
                    nc.vector.tensor_mul(oTf[r : r + 64, jc], o_sb[:], rb[:])

                # y projection for this t-block (needs both heads of J)
                for c in range(4):
                    y_ps = ps_y.tile([128, 512], F32, tag="y")
                    nc.tensor.matmul(
                        y_ps[:], w_p[:, ts(c, 128)], oTf[:, jc], start=True, stop=True
                    )
                    y_sb = ypool.tile([128, 512], F32, tag="ysb")
                    nc.vector.tensor_copy(y_sb[:], y_ps[:])
                    nc.sync.dma_start(y_d[ts(c, 128), jc], y_sb[:])

            if dbg:
                nc.sync.dma_start(dbg_qr[:], qrT[:])
                nc.sync.dma_start(dbg_kr[:], krT[:])
                va32 = big.tile([128, 2, NK, 65], F32)
                nc.vector.tensor_copy(va32[:], v_aug[:])
                nc.sync.dma_start(dbg_va[:], va32[:])
                nc.sync.dma_start(dbg_o[:], oTf[:])

    nc.compile()
    return nc


# ---------------- host-side wrapper ----------------

_CACHE = {}


def _get_nc(T):
    if T not in _CACHE:
        _CACHE[T] = build_kernel(T)
    return _CACHE[T]


def _swap_neg_rows(W):
    """Per 64-row head block of W [128, 512]: rows -> [W[32:64], -W[0:32]]."""
    out = np.empty_like(W)
    for h in range(2):
        r = 64 * h
        out[r : r + 32] = W[r + 32 : r + 64]
        out[r + 32 : r + 64] = -W[r : r + 32]
    return out


def _host_prep(x, cos, sin, Wq, Wk, Wv, Wp):
    T = x.shape[1]
    cosT = np.ascontiguousarray(cos.T).astype(np.float32)  # [32, T]
    sinT = np.ascontiguousarray(sin.T).astype(np.float32)
    ccT = np.concatenate([cosT] * 4, axis=0)  # [128, T]
    ssT = np.concatenate([sinT] * 4, axis=0)
    rr = np.arange(128)[:, None]
    cq = np.arange(512)[None, :]
    masks = np.stack(
        [(cq >= 128 * m + rr) for m in range(4)], axis=1
    ).astype(np.float16)  # [128, 4, 512]
    ident2 = np.concatenate([np.eye(64), np.eye(64)], axis=0).astype(np.float16)

    in_maps = []
    for core in range(N_CORES):
        b, p = core // 4, core % 4
        hs = slice(128 * p, 128 * (p + 1))
        Wq_h = Wq[hs].astype(np.float32)  # [128, 512]
        Wk_h = Wk[hs].astype(np.float32)
        in_maps.append(
            {
                "xT": np.ascontiguousarray(x[b].T.astype(np.float16)),
                "ccT": ccT,
                "ssT": ssT,
                "wqT": np.ascontiguousarray(Wq_h.T).astype(np.float16),
                "wqbT": np.ascontiguousarray(_swap_neg_rows(Wq_h).T).astype(np.float16),
                "wkT": np.ascontiguousarray(Wk_h.T).astype(np.float16),
                "wkbT": np.ascontiguousarray(_swap_neg_rows(Wk_h).T).astype(np.float16),
                "wvT": np.ascontiguousarray(Wv[hs].T.astype(np.float16)),
                "wpT": np.ascontiguousarray(Wp[:, hs].T.astype(np.float16)),
                "masks": masks,
                "ident2": ident2,
            }
        )
    return in_maps


def kernel(x, cos, sin, Wq, Wk, Wv, Wp, _trace=False, _nc=None):
    x = np.asarray(x)
    T = x.shape[1]
    nc = _nc if _nc is not None else _get_nc(T)
    in_maps = _host_prep(
        x, np.asarray(cos), np.asarray(sin),
        np.asarray(Wq), np.asarray(Wk), np.asarray(Wv), np.asarray(Wp),
    )
    res = run_bass_kernel_spmd(nc, in_maps, list(range(N_CORES)), trace=_trace)
    y = np.zeros((B, T, C), np.float32)
    for core in range(N_CORES):
        y[core // 4] += res.results[core]["yT"].T
    kernel.last_results = res
    return y
